# revision 7
# baseline (speedup 1.0000x reference)
"""Contrastive loss (SimCLR/NT-Xent style) kernel for Trainium2, 8 NeuronCores.

Reference computation:
    z   = l2_normalize(concat([emb_i, emb_j]))          # [2N, D] unit rows
    l   = (z @ z.T) / T                                 # [2N, 2N], T = 0.5
    lse = logsumexp(l with diag masked to -inf, axis=1)
    pos = l[i, (i + N) % 2N]
    loss = mean(lse - pos)

Strategy (per core c of 8; rows sharded):
    - Host concatenates + casts to bf16; every core loads the full
      [8192, 128] bf16 (2 MB) plus its own 1024-row slice and the positive-
      partner slice as separate per-core inputs (keeps the program static
      across cores).
    - All DMAs are issued up front; squared norms per chunk (square +
      free-dim reduce), then ONE Ln + ONE Exp computes rsqrt(s/2) =
      sqrt(2)/|e| for every row: the ACT table is loaded 3x total
      (ln, exp, tail ln) instead of thrashing per chunk.
    - z tiles (bf16, sqrt(2) folded) are transposed via PE (bf16 identity,
      single-pass) into ztall [128, 8192] / ztm [128, 1024].
    - Strips of 1536 cols per row-block: PE matmul -> PSUM f32, one ScalarE
      Exp with fused row-sum (accum_out).
    - Diag correction: subtract exp(|zt_i|^2) computed from the same bf16
      values the PE consumed; pos via multiply-reduce of bf16 z row tiles.
    - partial_c = sum over core rows of (log(S_i - exp(diag_i)) - pos_i);
      host sums 8 partials / 8192.
"""

import sys

if "/opt/trn_rl_repo" not in sys.path:
    sys.path.insert(0, "/opt/trn_rl_repo")

from contextlib import ExitStack

import numpy as np
import ml_dtypes

import concourse.bass as bass
import concourse.tile as tile
from concourse import bacc, mybir
from concourse.bass_utils import run_bass_kernel_spmd
from concourse.masks import make_identity

AF = mybir.ActivationFunctionType
ALU = mybir.AluOpType
AX = mybir.AxisListType
F32 = mybir.dt.float32
BF16 = mybir.dt.bfloat16

P = 128
N_CORES = 8


def build_program(R=8192, D=128, n_cores=N_CORES, strip_w=1536, chunk_t=16):
    """Builds the (static, SPMD) Bacc program run identically on all cores."""
    assert D == P
    rows_pc = R // n_cores
    assert rows_pc % P == 0
    mT = rows_pc // P  # row-blocks owned by this core
    T = R // P  # col tiles of the full matrix

    strips = []  # (col_off, width)
    off = 0
    while off < R:
        w = min(strip_w, R - off)
        strips.append((off, w))
        off += w
    S = len(strips)

    nc = bacc.Bacc(
        "TRN2",
        target_bir_lowering=False,
        debug=False,
        enable_asserts=False,
        num_devices=n_cores,
    )
    d_all = nc.dram_tensor("emb_all", [R, D], BF16, kind="ExternalInput")
    d_mine = nc.dram_tensor("emb_mine", [rows_pc, D], BF16, kind="ExternalInput")
    d_part = nc.dram_tensor("emb_partner", [rows_pc, D], BF16, kind="ExternalInput")
    d_out = nc.dram_tensor("partial", [1, 1], F32, kind="ExternalOutput")

    n_chunks = T // chunk_t

    with tile.TileContext(nc) as tc, ExitStack() as ctx:
        const_pool = ctx.enter_context(tc.tile_pool(name="const", bufs=1))
        persist = ctx.enter_context(tc.tile_pool(name="persist", bufs=1))
        sq_pool = ctx.enter_context(tc.tile_pool(name="sqp", bufs=2))
        zrow_pool = ctx.enter_context(tc.tile_pool(name="zrowp", bufs=8))
        ttr_pool = ctx.enter_context(tc.tile_pool(name="ttrp", bufs=2))
        psum_strip = ctx.enter_context(
            tc.tile_pool(name="psum_strip", bufs=2, space="PSUM")
        )
        psum_tp = ctx.enter_context(tc.tile_pool(name="psum_tp", bufs=2, space="PSUM"))

        identb = const_pool.tile([P, P], BF16, name="identb")
        make_identity(nc, identb[:])
        ones = const_pool.tile([P, 1], F32, name="ones")
        nc.gpsimd.memset(ones[:], 1.0)
        raw_all = persist.tile([P, T, P], BF16, name="raw_all")
        raw_mine = persist.tile([P, mT, P], BF16, name="raw_mine")
        raw_part = persist.tile([P, mT, P], BF16, name="raw_part")
        ztall = persist.tile([P, R], BF16, name="ztall")  # transposed z (rhs)
        ztm = persist.tile([P, rows_pc], BF16, name="ztm")  # transposed own (lhsT)
        zmine = persist.tile([P, mT, P], F32, name="zmine")  # own z rows
        zpart = persist.tile([P, mT, P], F32, name="zpart")  # partner z rows
        ssq = persist.tile([P, T + 2 * mT], F32, name="ssq")
        inv = persist.tile([P, T + 2 * mT], F32, name="inv")
        sums = persist.tile([P, S * mT], F32, name="sums")  # per (strip, rb)
        sqm = persist.tile([P, mT], F32, name="sqm")  # self-sim logits
        posv = persist.tile([P, mT], F32, name="posv")  # positive logits

        # --- DMA everything up front (mine first so its prep starts ASAP) ---
        # (p t) d layout: each partition reads a contiguous DRAM block, so
        # descriptors are KB-sized; rows land permuted (row p*tT+t at slot
        # (p, t)) which this loss is invariant to as long as mine/partner use
        # the same mapping. 16 parallel chunk DMAs issued from gpsimd (25ns
        # dispatch vs 565ns on sync) spread across DMA engines.
        nc.gpsimd.dma_start(
            raw_mine[:, :, :], d_mine[:, :].rearrange("(p t) d -> p t d", p=P)
        )
        nc.gpsimd.dma_start(
            raw_part[:, :, :], d_part[:, :].rearrange("(p t) d -> p t d", p=P)
        )
        all_src = d_all[:, :].rearrange("(p t) d -> p t d", p=P)
        dma_t = 4
        for c in range(T // dma_t):
            nc.gpsimd.dma_start(
                raw_all[:, c * dma_t : (c + 1) * dma_t, :],
                all_src[:, c * dma_t : (c + 1) * dma_t, :],
            )

        # --- squared row norms: square + free-dim reduce per chunk ---
        def emit_ssq(raw, tcount, col_off):
            sq = sq_pool.tile([P, chunk_t, P], BF16, name="sq", tag="sq")
            nc.vector.tensor_mul(sq[:, :tcount, :], raw, raw)
            nc.vector.reduce_sum(
                ssq[:, col_off : col_off + tcount], sq[:, :tcount, :], axis=AX.X
            )

        emit_ssq(raw_mine[:, :, :], mT, T)
        for c in range(n_chunks):
            emit_ssq(
                raw_all[:, c * chunk_t : (c + 1) * chunk_t, :], chunk_t, c * chunk_t
            )
        emit_ssq(raw_part[:, :, :], mT, T + mT)

        # rsqrt(s/2) as exp(-0.5*ln(s/2)): ONE Ln + ONE Exp for all rows
        lns = persist.tile([P, T + 2 * mT], F32, name="lns")
        nc.scalar.activation(lns[:, :], ssq[:, :], AF.Ln, scale=0.5)
        nc.scalar.activation(inv[:, :], lns[:, :], AF.Exp, scale=-0.5)

        # --- prep: scale (bf16, 2x mode) + transpose groups of 4 tiles ---
        def prep_group(raw, t0, tcount, inv_off, row_dst, zt_dst):
            tiles = []
            for k in range(tcount):
                c = inv_off + t0 + k
                if row_dst is not None:
                    nc.vector.tensor_scalar_mul(
                        row_dst[:, t0 + k, :], raw[:, t0 + k, :], inv[:, c : c + 1]
                    )
                if zt_dst is None:
                    continue
                zrt = zrow_pool.tile([P, P], BF16, name="zr", tag="zr")
                zr = zrt[:, :]
                nc.vector.tensor_scalar_mul(zr, raw[:, t0 + k, :], inv[:, c : c + 1])
                tiles.append(zr)
            if zt_dst is not None:
                tp = psum_tp.tile([P, 4 * P], BF16, name="tp", tag="tp")
                for k in range(tcount):
                    nc.tensor.transpose(tp[:, k * P : (k + 1) * P], tiles[k], identb[:])
                c0 = t0 * P
                nc.vector.tensor_copy(zt_dst[:, c0 : c0 + tcount * P], tp[:, : tcount * P])

        # own rows first (lhsT needed by every strip); zmine/sqm deferred
        for g in range(mT // 4):
            prep_group(raw_mine, g * 4, 4, T, None, ztm)

        def emit_deferred():
            # f32 z rows + self-sim + partner + positive logits: emitted inside
            # the strip phase so DVE does them while ACT chews exp strips
            for g in range(mT // 4):
                prep_group(raw_mine, g * 4, 4, T, zmine, None)
            for t in range(mT):
                tts = ttr_pool.tile([P, P], F32, name="tts", tag="tts")
                nc.vector.tensor_mul(tts[:, :], zmine[:, t, :], zmine[:, t, :])
                nc.vector.reduce_sum(sqm[:, t : t + 1], tts[:, :], axis=AX.X)
            for g in range(mT // 4):
                prep_group(raw_part, g * 4, 4, T + mT, zpart, None)
            for t in range(mT):
                ttp = ttr_pool.tile([P, P], F32, name="ttp", tag="tts")
                nc.vector.tensor_mul(ttp[:, :], zmine[:, t, :], zpart[:, t, :])
                nc.vector.reduce_sum(posv[:, t : t + 1], ttp[:, :], axis=AX.X)

        emitted = [0]

        def emit_all_until(tile_end):
            while emitted[0] < tile_end:
                t0 = emitted[0]
                cnt = min(4, T - t0)
                prep_group(raw_all, t0, cnt, 0, None, ztall)
                emitted[0] += cnt

        # --- strips: matmul -> Exp with fused row-sum (accum_out) ---
        for s, (c_off, w) in enumerate(strips):
            la_off, la_w = strips[min(s + 1, S - 1)]
            emit_all_until(min(T, (la_off + la_w + P - 1) // P))
            if s == 1:
                emit_deferred()
            for r in range(mT):
                ps = psum_strip.tile([P, w], F32, name="ps", tag="ps")
                m = 0
                while m < w:
                    mw = min(512, w - m)
                    nc.tensor.matmul(
                        ps[:, m : m + mw],
                        lhsT=ztm[:, r * P : (r + 1) * P],
                        rhs=ztall[:, c_off + m : c_off + m + mw],
                        start=True,
                        stop=True,
                    )
                    m += mw
                col = s * mT + r
                nc.scalar.activation(
                    ps[:, :], ps[:, :], AF.Exp, accum_out=sums[:, col : col + 1]
                )

        # --- tail: lse and loss partial ---
        sv = persist.tile([P, mT], F32, name="sv")
        nc.vector.reduce_sum(
            sv[:, :], sums[:].rearrange("p (s r) -> p r s", r=mT), axis=AX.X
        )
        expd = persist.tile([P, mT], F32, name="expd")
        nc.scalar.activation(expd[:, :], sqm[:, :], AF.Exp)
        sm = persist.tile([P, mT], F32, name="sm")
        nc.vector.tensor_sub(sm[:, :], sv[:, :], expd[:, :])
        lse = persist.tile([P, mT], F32, name="lse")
        nc.scalar.activation(lse[:, :], sm[:, :], AF.Ln)
        val = persist.tile([P, mT], F32, name="val")
        nc.vector.tensor_sub(val[:, :], lse[:, :], posv[:, :])
        val1 = persist.tile([P, 1], F32, name="val1")
        nc.vector.reduce_sum(val1[:, :], val[:, :], axis=AX.X)

        fps = psum_strip.tile([1, 1], F32, name="fps", tag="ps")
        nc.tensor.matmul(fps[:, :], lhsT=val1[:, :], rhs=ones[:, :], start=True, stop=True)
        res = persist.tile([1, 1], F32, name="res")
        nc.vector.tensor_copy(res[:, :], fps[:, :])
        nc.sync.dma_start(d_out[:, :], res[:, :])

    nc.compile()
    return nc


_CACHE = {}


def _get_program():
    if "nc" not in _CACHE:
        _CACHE["nc"] = build_program()
    return _CACHE["nc"]


def make_in_maps(emb_i, emb_j, n_cores=N_CORES):
    cat = np.ascontiguousarray(
        np.concatenate(
            [np.asarray(emb_i, np.float32), np.asarray(emb_j, np.float32)], axis=0
        )
    ).astype(ml_dtypes.bfloat16)
    R = cat.shape[0]
    rows_pc = R // n_cores
    in_maps = []
    for c in range(n_cores):
        lo = c * rows_pc
        plo = (lo + R // 2) % R
        in_maps.append(
            {
                "emb_all": cat,
                "emb_mine": np.ascontiguousarray(cat[lo : lo + rows_pc]),
                "emb_partner": np.ascontiguousarray(cat[plo : plo + rows_pc]),
            }
        )
    return in_maps


def kernel(emb_i, emb_j):
    nc = _get_program()
    in_maps = make_in_maps(emb_i, emb_j)
    results = run_bass_kernel_spmd(nc, in_maps, list(range(N_CORES))).results
    total = sum(float(results[c]["partial"][0, 0]) for c in range(N_CORES))
    R = np.asarray(emb_i).shape[0] * 2
    return np.float32(total / R)


# revision 10
# speedup vs baseline: 1.0020x; 1.0020x over previous
"""Contrastive loss (SimCLR/NT-Xent style) kernel for Trainium2, 8 NeuronCores.

Reference computation:
    z   = l2_normalize(concat([emb_i, emb_j]))          # [2N, D] unit rows
    l   = (z @ z.T) / T                                 # [2N, 2N], T = 0.5
    lse = logsumexp(l with diag masked to -inf, axis=1)
    pos = l[i, (i + N) % 2N]
    loss = mean(lse - pos)

Strategy (per core c of 8; rows sharded):
    - Host concatenates + casts to bf16; every core loads the full
      [8192, 128] bf16 (2 MB) plus its own 1024-row slice and the positive-
      partner slice as separate per-core inputs (keeps the program static
      across cores).
    - All DMAs are issued up front; squared norms per chunk (square +
      free-dim reduce), then ONE Ln + ONE Exp computes rsqrt(s/2) =
      sqrt(2)/|e| for every row: the ACT table is loaded 3x total
      (ln, exp, tail ln) instead of thrashing per chunk.
    - z tiles (bf16, sqrt(2) folded) are transposed via PE (bf16 identity,
      single-pass) into ztall [128, 8192] / ztm [128, 1024].
    - Strips of 1536 cols per row-block: PE matmul -> PSUM f32, one ScalarE
      Exp with fused row-sum (accum_out).
    - Diag correction: subtract exp(|zt_i|^2) computed from the same bf16
      values the PE consumed; pos via multiply-reduce of bf16 z row tiles.
    - partial_c = sum over core rows of (log(S_i - exp(diag_i)) - pos_i);
      host sums 8 partials / 8192.
"""

import sys

if "/opt/trn_rl_repo" not in sys.path:
    sys.path.insert(0, "/opt/trn_rl_repo")

from contextlib import ExitStack

import numpy as np
import ml_dtypes

import concourse.bass as bass
import concourse.tile as tile
from concourse import bacc, mybir
from concourse.bass_utils import run_bass_kernel_spmd
from concourse.masks import make_identity

AF = mybir.ActivationFunctionType
ALU = mybir.AluOpType
AX = mybir.AxisListType
F32 = mybir.dt.float32
BF16 = mybir.dt.bfloat16

P = 128
N_CORES = 8


def build_program(R=8192, D=128, n_cores=N_CORES, strip_w=1536, chunk_t=16):
    """Builds the (static, SPMD) Bacc program run identically on all cores."""
    assert D == P
    rows_pc = R // n_cores
    assert rows_pc % P == 0
    mT = rows_pc // P  # row-blocks owned by this core
    T = R // P  # col tiles of the full matrix

    strips = []  # (col_off, width)
    off = 0
    while off < R:
        w = min(strip_w, R - off)
        strips.append((off, w))
        off += w
    S = len(strips)

    nc = bacc.Bacc(
        "TRN2",
        target_bir_lowering=False,
        debug=False,
        enable_asserts=False,
        num_devices=n_cores,
    )
    d_all = nc.dram_tensor("emb_all", [R, D], BF16, kind="ExternalInput")
    d_mine = nc.dram_tensor("emb_mine", [rows_pc, D], BF16, kind="ExternalInput")
    d_part = nc.dram_tensor("emb_partner", [rows_pc, D], BF16, kind="ExternalInput")
    d_out = nc.dram_tensor("partial", [1, 1], F32, kind="ExternalOutput")

    n_chunks = T // chunk_t

    with tile.TileContext(nc) as tc, ExitStack() as ctx:
        const_pool = ctx.enter_context(tc.tile_pool(name="const", bufs=1))
        persist = ctx.enter_context(tc.tile_pool(name="persist", bufs=1))
        sq_pool = ctx.enter_context(tc.tile_pool(name="sqp", bufs=2))
        zrow_pool = ctx.enter_context(tc.tile_pool(name="zrowp", bufs=8))
        ttr_pool = ctx.enter_context(tc.tile_pool(name="ttrp", bufs=2))
        psum_strip = ctx.enter_context(
            tc.tile_pool(name="psum_strip", bufs=2, space="PSUM")
        )
        psum_tp = ctx.enter_context(tc.tile_pool(name="psum_tp", bufs=2, space="PSUM"))

        identb = const_pool.tile([P, P], BF16, name="identb")
        make_identity(nc, identb[:])
        ones = const_pool.tile([P, 1], F32, name="ones")
        nc.gpsimd.memset(ones[:], 1.0)
        raw_all = persist.tile([P, T, P], BF16, name="raw_all")
        raw_mine = persist.tile([P, mT, P], BF16, name="raw_mine")
        raw_part = persist.tile([P, mT, P], BF16, name="raw_part")
        ztall = persist.tile([P, R], BF16, name="ztall")  # transposed z (rhs)
        ztm = persist.tile([P, rows_pc], BF16, name="ztm")  # transposed own (lhsT)
        zmine = persist.tile([P, mT, P], F32, name="zmine")  # own z rows
        zpart = persist.tile([P, mT, P], F32, name="zpart")  # partner z rows
        ssq = persist.tile([P, T + 2 * mT], F32, name="ssq")
        inv = persist.tile([P, T + 2 * mT], F32, name="inv")
        sums = persist.tile([P, S * mT], F32, name="sums")  # per (strip, rb)
        sqm = persist.tile([P, mT], F32, name="sqm")  # self-sim logits
        posv = persist.tile([P, mT], F32, name="posv")  # positive logits

        # --- DMA everything up front ---
        # (p t) d layout: each partition reads a contiguous DRAM block, so
        # descriptors are KB-sized; rows land permuted (row p*tT+t at slot
        # (p, t)) which this loss is invariant to as long as mine/partner use
        # the same mapping. Issues are spread across four engine queues so
        # the ~600ns per-issue sequencer cost overlaps; batch-1 data (mine +
        # tiles 0-15) is issued first on each queue.
        all_src = d_all[:, :].rearrange("(p t) d -> p t d", p=P)
        dma_t = 4

        def chunk_dma(eng, c):
            eng.dma_start(
                raw_all[:, c * dma_t : (c + 1) * dma_t, :],
                all_src[:, c * dma_t : (c + 1) * dma_t, :],
            )

        nc.sync.dma_start(
            raw_mine[:, :, :], d_mine[:, :].rearrange("(p t) d -> p t d", p=P)
        )
        chunk_dma(nc.scalar, 0)
        chunk_dma(nc.gpsimd, 1)
        chunk_dma(nc.gpsimd, 2)
        chunk_dma(nc.sync, 3)
        for c in range(4, 16):
            chunk_dma([nc.scalar, nc.gpsimd, nc.sync][c % 3], c)
        nc.gpsimd.dma_start(
            raw_part[:, :, :], d_part[:, :].rearrange("(p t) d -> p t d", p=P)
        )

        # --- squared row norms: square + free-dim reduce per group ---
        def emit_ssq(raw, tcount, col_off):
            sq = sq_pool.tile([P, chunk_t, P], BF16, name="sq", tag="sq")
            nc.vector.tensor_mul(sq[:, :tcount, :], raw, raw)
            nc.vector.reduce_sum(
                ssq[:, col_off : col_off + tcount], sq[:, :tcount, :], axis=AX.X
            )

        lns = persist.tile([P, T + 2 * mT], F32, name="lns")

        def emit_inv(c0, c1):
            # rsqrt(s/2) as exp(-0.5*ln(s/2)); Ln/Exp batched per range so the
            # ACT table switches ln->exp once per batch
            nc.scalar.activation(lns[:, c0:c1], ssq[:, c0:c1], AF.Ln, scale=0.5)
            nc.scalar.activation(inv[:, c0:c1], lns[:, c0:c1], AF.Exp, scale=-0.5)

        def emit_inv2(r0, r1):
            nc.scalar.activation(lns[:, r0[0]:r0[1]], ssq[:, r0[0]:r0[1]], AF.Ln, scale=0.5)
            nc.scalar.activation(lns[:, r1[0]:r1[1]], ssq[:, r1[0]:r1[1]], AF.Ln, scale=0.5)
            nc.scalar.activation(inv[:, r0[0]:r0[1]], lns[:, r0[0]:r0[1]], AF.Exp, scale=-0.5)
            nc.scalar.activation(inv[:, r1[0]:r1[1]], lns[:, r1[0]:r1[1]], AF.Exp, scale=-0.5)

        # batch 1: mine + tiles 0-15 -> inv ready early, strips start ~7us
        emit_ssq(raw_mine[:, :, :], mT, T)
        emit_ssq(raw_all[:, 0:chunk_t, :], chunk_t, 0)
        emit_inv2((0, chunk_t), (T, T + mT))

        def emit_batch2_norms():
            for c in range(1, n_chunks):
                emit_ssq(
                    raw_all[:, c * chunk_t : (c + 1) * chunk_t, :], chunk_t, c * chunk_t
                )
            emit_ssq(raw_part[:, :, :], mT, T + mT)
            emit_inv2((chunk_t, T), (T + mT, T + 2 * mT))

        # --- prep: scale (bf16, 2x mode) + transpose groups of 4 tiles ---
        def prep_group(raw, t0, tcount, inv_off, row_dst, zt_dst):
            tiles = []
            for k in range(tcount):
                c = inv_off + t0 + k
                if row_dst is not None:
                    nc.vector.tensor_scalar_mul(
                        row_dst[:, t0 + k, :], raw[:, t0 + k, :], inv[:, c : c + 1]
                    )
                if zt_dst is None:
                    continue
                zrt = zrow_pool.tile([P, P], BF16, name="zr", tag="zr")
                zr = zrt[:, :]
                nc.vector.tensor_scalar_mul(zr, raw[:, t0 + k, :], inv[:, c : c + 1])
                tiles.append(zr)
            if zt_dst is not None:
                tp = psum_tp.tile([P, 4 * P], BF16, name="tp", tag="tp")
                for k in range(tcount):
                    nc.tensor.transpose(tp[:, k * P : (k + 1) * P], tiles[k], identb[:])
                c0 = t0 * P
                nc.vector.tensor_copy(zt_dst[:, c0 : c0 + tcount * P], tp[:, : tcount * P])

        # own rows first (lhsT needed by every strip); zmine/sqm deferred
        for g in range(mT // 4):
            prep_group(raw_mine, g * 4, 4, T, None, ztm)

        def emit_deferred():
            # f32 z rows + self-sim + partner + positive logits: emitted inside
            # the strip phase so DVE does them while ACT chews exp strips
            for g in range(mT // 4):
                prep_group(raw_mine, g * 4, 4, T, zmine, None)
            for t in range(mT):
                tts = ttr_pool.tile([P, P], F32, name="tts", tag="tts")
                nc.vector.tensor_mul(tts[:, :], zmine[:, t, :], zmine[:, t, :])
                nc.vector.reduce_sum(sqm[:, t : t + 1], tts[:, :], axis=AX.X)
            for g in range(mT // 4):
                prep_group(raw_part, g * 4, 4, T + mT, zpart, None)
            for t in range(mT):
                ttp = ttr_pool.tile([P, P], F32, name="ttp", tag="tts")
                nc.vector.tensor_mul(ttp[:, :], zmine[:, t, :], zpart[:, t, :])
                nc.vector.reduce_sum(posv[:, t : t + 1], ttp[:, :], axis=AX.X)

        emitted = [0]

        def emit_all_until(tile_end):
            while emitted[0] < tile_end:
                t0 = emitted[0]
                cnt = min(4, T - t0)
                prep_group(raw_all, t0, cnt, 0, None, ztall)
                emitted[0] += cnt

        # --- strips: matmul -> Exp with fused row-sum (accum_out) ---
        for s, (c_off, w) in enumerate(strips):
            if s == 1:
                emit_batch2_norms()
                emit_deferred()
            if s == 0:
                emit_all_until(min(T, (c_off + w + P - 1) // P))
            else:
                la_off, la_w = strips[min(s + 1, S - 1)]
                emit_all_until(min(T, (la_off + la_w + P - 1) // P))
            for r in range(mT):
                ps = psum_strip.tile([P, w], F32, name="ps", tag="ps")
                m = 0
                while m < w:
                    mw = min(512, w - m)
                    nc.tensor.matmul(
                        ps[:, m : m + mw],
                        lhsT=ztm[:, r * P : (r + 1) * P],
                        rhs=ztall[:, c_off + m : c_off + m + mw],
                        start=True,
                        stop=True,
                    )
                    m += mw
                col = s * mT + r
                nc.scalar.activation(
                    ps[:, :], ps[:, :], AF.Exp, accum_out=sums[:, col : col + 1]
                )

        # --- tail: lse and loss partial ---
        sv = persist.tile([P, mT], F32, name="sv")
        nc.vector.reduce_sum(
            sv[:, :], sums[:].rearrange("p (s r) -> p r s", r=mT), axis=AX.X
        )
        expd = persist.tile([P, mT], F32, name="expd")
        nc.scalar.activation(expd[:, :], sqm[:, :], AF.Exp)
        sm = persist.tile([P, mT], F32, name="sm")
        nc.vector.tensor_sub(sm[:, :], sv[:, :], expd[:, :])
        lse = persist.tile([P, mT], F32, name="lse")
        nc.scalar.activation(lse[:, :], sm[:, :], AF.Ln)
        val = persist.tile([P, mT], F32, name="val")
        nc.vector.tensor_sub(val[:, :], lse[:, :], posv[:, :])
        val1 = persist.tile([P, 1], F32, name="val1")
        nc.vector.reduce_sum(val1[:, :], val[:, :], axis=AX.X)

        fps = psum_strip.tile([1, 1], F32, name="fps", tag="ps")
        nc.tensor.matmul(fps[:, :], lhsT=val1[:, :], rhs=ones[:, :], start=True, stop=True)
        res = persist.tile([1, 1], F32, name="res")
        nc.vector.tensor_copy(res[:, :], fps[:, :])
        nc.sync.dma_start(d_out[:, :], res[:, :])

    nc.compile()
    return nc


_CACHE = {}


def _get_program():
    if "nc" not in _CACHE:
        _CACHE["nc"] = build_program()
    return _CACHE["nc"]


def make_in_maps(emb_i, emb_j, n_cores=N_CORES):
    cat = np.ascontiguousarray(
        np.concatenate(
            [np.asarray(emb_i, np.float32), np.asarray(emb_j, np.float32)], axis=0
        )
    ).astype(ml_dtypes.bfloat16)
    R = cat.shape[0]
    rows_pc = R // n_cores
    in_maps = []
    for c in range(n_cores):
        lo = c * rows_pc
        plo = (lo + R // 2) % R
        in_maps.append(
            {
                "emb_all": cat,
                "emb_mine": np.ascontiguousarray(cat[lo : lo + rows_pc]),
                "emb_partner": np.ascontiguousarray(cat[plo : plo + rows_pc]),
            }
        )
    return in_maps


def kernel(emb_i, emb_j):
    nc = _get_program()
    in_maps = make_in_maps(emb_i, emb_j)
    results = run_bass_kernel_spmd(nc, in_maps, list(range(N_CORES))).results
    total = sum(float(results[c]["partial"][0, 0]) for c in range(N_CORES))
    R = np.asarray(emb_i).shape[0] * 2
    return np.float32(total / R)


# revision 11
# speedup vs baseline: 1.0589x; 1.0568x over previous
"""Contrastive loss (SimCLR/NT-Xent style) kernel for Trainium2, 8 NeuronCores.

Reference computation:
    z   = l2_normalize(concat([emb_i, emb_j]))          # [2N, D] unit rows
    l   = (z @ z.T) / T                                 # [2N, 2N], T = 0.5
    lse = logsumexp(l with diag masked to -inf, axis=1)
    pos = l[i, (i + N) % 2N]
    loss = mean(lse - pos)

Strategy (per core c of 8; rows sharded):
    - Host concatenates + casts to bf16; every core loads the full
      [8192, 128] bf16 (2 MB) plus its own 1024-row slice and the positive-
      partner slice as separate per-core inputs (keeps the program static
      across cores).
    - All DMAs are issued up front; squared norms per chunk (square +
      free-dim reduce), then ONE Ln + ONE Exp computes rsqrt(s/2) =
      sqrt(2)/|e| for every row: the ACT table is loaded 3x total
      (ln, exp, tail ln) instead of thrashing per chunk.
    - z tiles (bf16, sqrt(2) folded) are transposed via PE (bf16 identity,
      single-pass) into ztall [128, 8192] / ztm [128, 1024].
    - Strips of 1536 cols per row-block: PE matmul -> PSUM f32, one ScalarE
      Exp with fused row-sum (accum_out).
    - Diag correction: subtract exp(|zt_i|^2) computed from the same bf16
      values the PE consumed; pos via multiply-reduce of bf16 z row tiles.
    - partial_c = sum over core rows of (log(S_i - exp(diag_i)) - pos_i);
      host sums 8 partials / 8192.
"""

import sys

if "/opt/trn_rl_repo" not in sys.path:
    sys.path.insert(0, "/opt/trn_rl_repo")

from contextlib import ExitStack

import numpy as np
import ml_dtypes

import concourse.bass as bass
import concourse.tile as tile
from concourse import bacc, mybir
from concourse.bass_utils import run_bass_kernel_spmd
from concourse.masks import make_identity

AF = mybir.ActivationFunctionType
ALU = mybir.AluOpType
AX = mybir.AxisListType
F32 = mybir.dt.float32
BF16 = mybir.dt.bfloat16

P = 128
N_CORES = 8


def build_program(R=8192, D=128, n_cores=N_CORES, strip_w=1536, chunk_t=16):
    """Builds the (static, SPMD) Bacc program run identically on all cores."""
    assert D == P
    rows_pc = R // n_cores
    assert rows_pc % P == 0
    mT = rows_pc // P  # row-blocks owned by this core
    T = R // P  # col tiles of the full matrix

    strips = []  # (col_off, width)
    off = 0
    while off < R:
        w = min(strip_w, R - off)
        strips.append((off, w))
        off += w
    S = len(strips)

    nc = bacc.Bacc(
        "TRN2",
        target_bir_lowering=False,
        debug=False,
        enable_asserts=False,
        num_devices=n_cores,
    )
    d_all = nc.dram_tensor("emb_all", [R, D], BF16, kind="ExternalInput")
    d_mine = nc.dram_tensor("emb_mine", [rows_pc, D], BF16, kind="ExternalInput")
    d_part = nc.dram_tensor("emb_partner", [rows_pc, D], BF16, kind="ExternalInput")
    d_out = nc.dram_tensor("partial", [1, 1], F32, kind="ExternalOutput")

    n_chunks = T // chunk_t

    with tile.TileContext(nc) as tc, ExitStack() as ctx:
        const_pool = ctx.enter_context(tc.tile_pool(name="const", bufs=1))
        persist = ctx.enter_context(tc.tile_pool(name="persist", bufs=1))
        sq_pool = ctx.enter_context(tc.tile_pool(name="sqp", bufs=2))
        zrow_pool = ctx.enter_context(tc.tile_pool(name="zrowp", bufs=8))
        ttr_pool = ctx.enter_context(tc.tile_pool(name="ttrp", bufs=2))
        psum_strip = ctx.enter_context(
            tc.tile_pool(name="psum_strip", bufs=2, space="PSUM")
        )
        psum_tp = ctx.enter_context(tc.tile_pool(name="psum_tp", bufs=2, space="PSUM"))

        identb = const_pool.tile([P, P], BF16, name="identb")
        make_identity(nc, identb[:])
        ones = const_pool.tile([P, 1], F32, name="ones")
        nc.gpsimd.memset(ones[:], 1.0)
        raw_all = persist.tile([P, T, P], BF16, name="raw_all")
        raw_mine = persist.tile([P, mT, P], BF16, name="raw_mine")
        raw_part = persist.tile([P, mT, P], BF16, name="raw_part")
        ztall = persist.tile([P, R], BF16, name="ztall")  # transposed z (rhs)
        ztm = persist.tile([P, rows_pc], BF16, name="ztm")  # transposed own (lhsT)
        zmine = persist.tile([P, mT, P], F32, name="zmine")  # own z rows
        zpart = persist.tile([P, mT, P], F32, name="zpart")  # partner z rows
        ssq = persist.tile([P, T + 2 * mT], F32, name="ssq")
        inv = persist.tile([P, T + 2 * mT], F32, name="inv")
        sums = persist.tile([P, S * mT], F32, name="sums")  # per (strip, rb)
        sqm = persist.tile([P, mT], F32, name="sqm")  # self-sim logits
        posv = persist.tile([P, mT], F32, name="posv")  # positive logits

        # --- DMA everything up front ---
        # (p t) d layout: each partition reads a contiguous DRAM block, so
        # descriptors are KB-sized; rows land permuted (row p*tT+t at slot
        # (p, t)) which this loss is invariant to as long as mine/partner use
        # the same mapping. Issues are spread across four engine queues so
        # the ~600ns per-issue sequencer cost overlaps; batch-1 data (mine +
        # tiles 0-15) is issued first on each queue.
        all_src = d_all[:, :].rearrange("(p t) d -> p t d", p=P)
        dma_t = 4

        def chunk_dma(eng, c):
            eng.dma_start(
                raw_all[:, c * dma_t : (c + 1) * dma_t, :],
                all_src[:, c * dma_t : (c + 1) * dma_t, :],
            )

        nc.sync.dma_start(
            raw_mine[:, :, :], d_mine[:, :].rearrange("(p t) d -> p t d", p=P)
        )
        chunk_dma(nc.scalar, 0)
        chunk_dma(nc.gpsimd, 1)
        chunk_dma(nc.gpsimd, 2)
        chunk_dma(nc.sync, 3)
        for c in range(4, 16):
            chunk_dma([nc.scalar, nc.gpsimd, nc.sync][c % 3], c)
        nc.gpsimd.dma_start(
            raw_part[:, :, :], d_part[:, :].rearrange("(p t) d -> p t d", p=P)
        )

        # --- squared row norms: square + free-dim reduce per group ---
        def emit_ssq(raw, tcount, col_off):
            sq = sq_pool.tile([P, chunk_t, P], BF16, name="sq", tag="sq")
            nc.vector.tensor_mul(sq[:, :tcount, :], raw, raw)
            nc.vector.reduce_sum(
                ssq[:, col_off : col_off + tcount], sq[:, :tcount, :], axis=AX.X
            )

        lns = persist.tile([P, T + 2 * mT], F32, name="lns")

        def emit_inv(c0, c1):
            # rsqrt(s/2) as exp(-0.5*ln(s/2)); Ln/Exp batched per range so the
            # ACT table switches ln->exp once per batch
            nc.scalar.activation(lns[:, c0:c1], ssq[:, c0:c1], AF.Ln, scale=0.5)
            nc.scalar.activation(inv[:, c0:c1], lns[:, c0:c1], AF.Exp, scale=-0.5)

        def emit_inv2(r0, r1):
            nc.scalar.activation(lns[:, r0[0]:r0[1]], ssq[:, r0[0]:r0[1]], AF.Ln, scale=0.5)
            nc.scalar.activation(lns[:, r1[0]:r1[1]], ssq[:, r1[0]:r1[1]], AF.Ln, scale=0.5)
            nc.scalar.activation(inv[:, r0[0]:r0[1]], lns[:, r0[0]:r0[1]], AF.Exp, scale=-0.5)
            nc.scalar.activation(inv[:, r1[0]:r1[1]], lns[:, r1[0]:r1[1]], AF.Exp, scale=-0.5)

        # batch 1: mine + tiles 0-15 -> inv ready early, strips start ~7us
        emit_ssq(raw_mine[:, :, :], mT, T)
        emit_ssq(raw_all[:, 0:chunk_t, :], chunk_t, 0)
        emit_inv2((0, chunk_t), (T, T + mT))

        def emit_batch2_norms():
            for c in range(1, n_chunks):
                emit_ssq(
                    raw_all[:, c * chunk_t : (c + 1) * chunk_t, :], chunk_t, c * chunk_t
                )
            emit_ssq(raw_part[:, :, :], mT, T + mT)
            emit_inv2((chunk_t, T), (T + mT, T + 2 * mT))

        # --- prep: scale (bf16, 2x mode) + transpose groups of 4 tiles ---
        def prep_group(raw, t0, tcount, inv_off, row_dst, zt_dst):
            tiles = []
            for k in range(tcount):
                c = inv_off + t0 + k
                if row_dst is not None:
                    nc.vector.tensor_scalar_mul(
                        row_dst[:, t0 + k, :], raw[:, t0 + k, :], inv[:, c : c + 1]
                    )
                if zt_dst is None:
                    continue
                zrt = zrow_pool.tile([P, P], BF16, name="zr", tag="zr")
                zr = zrt[:, :]
                nc.vector.tensor_scalar_mul(zr, raw[:, t0 + k, :], inv[:, c : c + 1])
                tiles.append(zr)
            if zt_dst is not None:
                tp = psum_tp.tile([P, 4 * P], BF16, name="tp", tag="tp")
                for k in range(tcount):
                    nc.tensor.transpose(tp[:, k * P : (k + 1) * P], tiles[k], identb[:])
                c0 = t0 * P
                nc.vector.tensor_copy(zt_dst[:, c0 : c0 + tcount * P], tp[:, : tcount * P])

        # own rows first (lhsT needed by every strip); zmine/sqm deferred
        for g in range(mT // 4):
            prep_group(raw_mine, g * 4, 4, T, None, ztm)

        def emit_deferred():
            # f32 z rows + self-sim + partner + positive logits: emitted inside
            # the strip phase so DVE does them while ACT chews exp strips
            for g in range(mT // 4):
                prep_group(raw_mine, g * 4, 4, T, zmine, None)
            for t in range(mT):
                tts = ttr_pool.tile([P, P], F32, name="tts", tag="tts")
                nc.vector.tensor_mul(tts[:, :], zmine[:, t, :], zmine[:, t, :])
                nc.vector.reduce_sum(sqm[:, t : t + 1], tts[:, :], axis=AX.X)
            for g in range(mT // 4):
                prep_group(raw_part, g * 4, 4, T + mT, zpart, None)
            for t in range(mT):
                ttp = ttr_pool.tile([P, P], F32, name="ttp", tag="tts")
                nc.vector.tensor_mul(ttp[:, :], zmine[:, t, :], zpart[:, t, :])
                nc.vector.reduce_sum(posv[:, t : t + 1], ttp[:, :], axis=AX.X)

        emitted = [0]

        def emit_all_until(tile_end):
            while emitted[0] < tile_end:
                t0 = emitted[0]
                cnt = min(4, T - t0)
                prep_group(raw_all, t0, cnt, 0, None, ztall)
                emitted[0] += cnt

        # --- strips: matmul -> Exp with fused row-sum (accum_out) ---
        for s, (c_off, w) in enumerate(strips):
            if s == 0:
                # batch-2 squares/inv only need DMA'd raw data; DVE does them
                # while ACT chews strip 0 (emitted before strip 0's prep would
                # be wrong: prep of tiles 12+ must come after inv2)
                pass
            if s == 1:
                emit_batch2_norms()
            if s == 3:
                emit_deferred()
            if s == 0:
                emit_all_until(min(T, (c_off + w + P - 1) // P))
            else:
                la_off, la_w = strips[min(s + 1, S - 1)]
                emit_all_until(min(T, (la_off + la_w + P - 1) // P))
            for r in range(mT):
                ps = psum_strip.tile([P, w], F32, name="ps", tag="ps")
                m = 0
                while m < w:
                    mw = min(512, w - m)
                    nc.tensor.matmul(
                        ps[:, m : m + mw],
                        lhsT=ztm[:, r * P : (r + 1) * P],
                        rhs=ztall[:, c_off + m : c_off + m + mw],
                        start=True,
                        stop=True,
                    )
                    m += mw
                col = s * mT + r
                nc.scalar.activation(
                    ps[:, :], ps[:, :], AF.Exp, accum_out=sums[:, col : col + 1]
                )

        # --- tail: lse and loss partial ---
        sv = persist.tile([P, mT], F32, name="sv")
        nc.vector.reduce_sum(
            sv[:, :], sums[:].rearrange("p (s r) -> p r s", r=mT), axis=AX.X
        )
        expd = persist.tile([P, mT], F32, name="expd")
        nc.scalar.activation(expd[:, :], sqm[:, :], AF.Exp)
        sm = persist.tile([P, mT], F32, name="sm")
        nc.vector.tensor_sub(sm[:, :], sv[:, :], expd[:, :])
        lse = persist.tile([P, mT], F32, name="lse")
        nc.scalar.activation(lse[:, :], sm[:, :], AF.Ln)
        val = persist.tile([P, mT], F32, name="val")
        nc.vector.tensor_sub(val[:, :], lse[:, :], posv[:, :])
        val1 = persist.tile([P, 1], F32, name="val1")
        nc.vector.reduce_sum(val1[:, :], val[:, :], axis=AX.X)

        fps = psum_strip.tile([1, 1], F32, name="fps", tag="ps")
        nc.tensor.matmul(fps[:, :], lhsT=val1[:, :], rhs=ones[:, :], start=True, stop=True)
        res = persist.tile([1, 1], F32, name="res")
        nc.vector.tensor_copy(res[:, :], fps[:, :])
        nc.sync.dma_start(d_out[:, :], res[:, :])

    nc.compile()
    return nc


_CACHE = {}


def _get_program():
    if "nc" not in _CACHE:
        _CACHE["nc"] = build_program()
    return _CACHE["nc"]


def make_in_maps(emb_i, emb_j, n_cores=N_CORES):
    cat = np.ascontiguousarray(
        np.concatenate(
            [np.asarray(emb_i, np.float32), np.asarray(emb_j, np.float32)], axis=0
        )
    ).astype(ml_dtypes.bfloat16)
    R = cat.shape[0]
    rows_pc = R // n_cores
    in_maps = []
    for c in range(n_cores):
        lo = c * rows_pc
        plo = (lo + R // 2) % R
        in_maps.append(
            {
                "emb_all": cat,
                "emb_mine": np.ascontiguousarray(cat[lo : lo + rows_pc]),
                "emb_partner": np.ascontiguousarray(cat[plo : plo + rows_pc]),
            }
        )
    return in_maps


def kernel(emb_i, emb_j):
    nc = _get_program()
    in_maps = make_in_maps(emb_i, emb_j)
    results = run_bass_kernel_spmd(nc, in_maps, list(range(N_CORES))).results
    total = sum(float(results[c]["partial"][0, 0]) for c in range(N_CORES))
    R = np.asarray(emb_i).shape[0] * 2
    return np.float32(total / R)
